# revision 1
# baseline (speedup 1.0000x reference)
"""DeepSeek-V3 MoE routing kernel for Trainium2 (Bass/Tile), 8-core SPMD.

Group-top2 phase restructured around the exact pair-tree identity:

  top2sum(S) = max( max_i (a_i + b_i),  top2sum(pairmax) )

applied recursively over pair levels 32->16->8->4->2->1.  This is exact and
tie-safe (duplicated maxima give 2*max via the pair sums).  Measured Pool
tensor_tensor runs at ~0.4 ns/elem on hardware, so the whole tree (~91
elem-ops per group of 32) is cheap there, while the DVE keeps only the
top-8 extraction (Max8/MaxIndex) plus narrow group-rank ops.

Engine split: ACT = sigmoid + sign-select; Pool = swb-add, pair-tree,
swbm mask-add, s-mult, vals scale; DVE = Max8/MaxIndex, narrow rank ops.
"""

import numpy as np

T_FULL = 131072
E = 256
G = 8
EG = 32
N_CORES = 8
T_CORE = T_FULL // N_CORES
P = 128
NEG = -1.0e30
TB = 8  # tiles per batch group
SPLIT = 2  # tiles per Pool sub-op for swbm/s (pipelining granularity)
SM_DVE_CHUNKS = 0  # s-mult chunks (of TB//SPLIT) on DVE instead of Pool


def build_bass(n_tokens: int):
    from contextlib import ExitStack

    import concourse.bacc as bacc
    import concourse.mybir as mybir
    import concourse.tile as tile

    f32 = mybir.dt.float32
    A = mybir.AluOpType
    AX = mybir.AxisListType
    AF = mybir.ActivationFunctionType

    assert n_tokens % (P * TB) == 0
    n_groups = n_tokens // (P * TB)
    W = TB * E
    JG = TB * G

    SW = SPLIT * E
    nc = bacc.Bacc("TRN2", target_bir_lowering=False, debug=False)

    logits_d = nc.dram_tensor("logits", [n_tokens, E], f32, kind="ExternalInput").ap()
    biasb_d = nc.dram_tensor("biasb", [P, W], f32, kind="ExternalInput").ap()
    idx_d = nc.dram_tensor("idx", [n_tokens, 8], mybir.dt.int32, kind="ExternalOutput").ap()
    vals_d = nc.dram_tensor("vals", [n_tokens, 8], f32, kind="ExternalOutput").ap()

    with tile.TileContext(nc) as tc, ExitStack() as ctx:
        setup = ctx.enter_context(tc.tile_pool(name="setup", bufs=1))
        big = ctx.enter_context(tc.tile_pool(name="big", bufs=4))
        med = ctx.enter_context(tc.tile_pool(name="med", bufs=2))
        small = ctx.enter_context(tc.tile_pool(name="small", bufs=3))

        bias_bc = setup.tile([P, W], f32)
        nc.sync.dma_start(bias_bc[:], biasb_d)
        negc = setup.tile([P, 1], f32)
        nc.vector.memset(negc[:], NEG)

        def phase_a1(i):
            """Load, sigmoid, biased add, pair-tree group scores, group
            mask; ends issuing the Pool swbm adds."""
            rows = slice(i * P * TB, (i + 1) * P * TB)
            dview = logits_d[rows, :].rearrange("(j p) e -> p j e", p=P)

            scores = big.tile([P, W], f32, tag="scores")
            swb = big.tile([P, W], f32, tag="swb")

            if i == 0:
                for j in range(0, TB, SPLIT):
                    sl = slice(j * E, j * E + SW)
                    nc.sync.dma_start(
                        scores[:, sl].rearrange("p (j e) -> p j e", j=SPLIT),
                        dview[:, j:j + SPLIT, :])
                    nc.scalar.activation(scores[:, sl], scores[:, sl], AF.Sigmoid)
                    nc.gpsimd.tensor_add(swb[:, sl], scores[:, sl],
                                         bias_bc[:, sl])
            else:
                nc.sync.dma_start(scores[:].rearrange("p (j e) -> p j e", j=TB),
                                  dview)
                nc.scalar.activation(scores[:], scores[:], AF.Sigmoid)
                nc.gpsimd.tensor_add(swb[:], scores[:], bias_bc[:])

            # --- pair-tree group scores (hybrid) ---
            # top2sum(S) = max over pair-sums at all levels; pair-sums (adds)
            # run on Pool (supports gapped APs), pair-maxes on DVE, and the
            # final per-group max over the concatenated 31 pair-sums is one
            # grouped DVE reduce.
            u16 = med.tile([P, JG * 16], f32, tag="u16")
            u8 = small.tile([P, JG * 8], f32, tag="u8")
            u4 = small.tile([P, JG * 4], f32, tag="u4")
            u2 = small.tile([P, JG * 2], f32, tag="u2")
            pcat = med.tile([P, JG * 23], f32, tag="pcat")
            pv = pcat[:].rearrange("p (jg c) -> p jg c", jg=JG)
            p16s = med.tile([P, JG * 16], f32, tag="p16s")
            p16v = p16s[:].rearrange("p (jg c) -> p jg c", jg=JG)

            def halves(t, n):
                v = t[:].rearrange("p (jg two c) -> p jg two c", jg=JG, two=2)
                return v[:, :, 0, :], v[:, :, 1, :]

            def uview(t):
                return t[:].rearrange("p (jg c) -> p jg c", jg=JG)

            a, b = halves(swb, 16)
            d16 = med.tile([P, JG * 16], f32, tag="d16")
            nc.gpsimd.tensor_tensor(uview(d16), a, b, op=A.subtract)
            nc.scalar.activation(d16[:], d16[:], AF.Abs, bias=0.0, scale=0.5)
            nc.gpsimd.tensor_tensor(p16v, a, b, op=A.add)
            # u16 = (a+b)/2 + |a-b|/2
            nc.gpsimd.tensor_scalar(uview(u16), p16v, 0.5, None, op0=A.mult)
            nc.gpsimd.tensor_tensor(u16[:], u16[:], d16[:], op=A.add)
            # z8 = max(p16 halves) via the same identity -> pcat slot [15:23]
            pa, pb = halves(p16s, 8)
            dz = small.tile([P, JG * 8], f32, tag="dz")
            nc.gpsimd.tensor_tensor(uview(dz), pa, pb, op=A.subtract)
            nc.scalar.activation(dz[:], dz[:], AF.Abs, bias=0.0, scale=0.5)
            nc.gpsimd.tensor_tensor(pv[:, :, 15:23], pa, pb, op=A.add)
            nc.gpsimd.tensor_scalar(pv[:, :, 15:23], pv[:, :, 15:23], 0.5, None, op0=A.mult)
            nc.gpsimd.tensor_tensor(
                pv[:, :, 15:23], pv[:, :, 15:23],
                dz[:].rearrange("p (jg c) -> p jg c", jg=JG), op=A.add)
            a, b = halves(u16, 8)
            d8 = small.tile([P, JG * 8], f32, tag="d8")
            nc.gpsimd.tensor_tensor(uview(d8), a, b, op=A.subtract)
            nc.scalar.activation(d8[:], d8[:], AF.Abs, bias=0.0, scale=0.5)
            nc.gpsimd.tensor_tensor(pv[:, :, 0:8], a, b, op=A.add)
            nc.gpsimd.tensor_scalar(uview(u8), pv[:, :, 0:8], 0.5, None, op0=A.mult)
            nc.gpsimd.tensor_tensor(u8[:], u8[:], d8[:], op=A.add)
            a, b = halves(u8, 4)
            d4 = small.tile([P, JG * 4], f32, tag="d4")
            nc.gpsimd.tensor_tensor(uview(d4), a, b, op=A.subtract)
            nc.scalar.activation(d4[:], d4[:], AF.Abs, bias=0.0, scale=0.5)
            nc.gpsimd.tensor_tensor(pv[:, :, 8:12], a, b, op=A.add)
            nc.gpsimd.tensor_scalar(uview(u4), pv[:, :, 8:12], 0.5, None, op0=A.mult)
            nc.gpsimd.tensor_tensor(u4[:], u4[:], d4[:], op=A.add)
            a, b = halves(u4, 2)
            d2 = small.tile([P, JG * 2], f32, tag="d2")
            nc.gpsimd.tensor_tensor(uview(d2), a, b, op=A.subtract)
            nc.scalar.activation(d2[:], d2[:], AF.Abs, bias=0.0, scale=0.5)
            nc.gpsimd.tensor_tensor(pv[:, :, 12:14], a, b, op=A.add)
            nc.gpsimd.tensor_scalar(uview(u2), pv[:, :, 12:14], 0.5, None, op0=A.mult)
            nc.gpsimd.tensor_tensor(u2[:], u2[:], d2[:], op=A.add)
            a, b = halves(u2, 1)
            nc.gpsimd.tensor_tensor(pv[:, :, 14:15], a, b, op=A.add)

            # fold z8 slot into p8 slot: y = max(p8, z8) via the abs identity
            dy = small.tile([P, JG * 8], f32, tag="dy")
            nc.gpsimd.tensor_tensor(
                dy[:].rearrange("p (jg c) -> p jg c", jg=JG),
                pv[:, :, 0:8], pv[:, :, 15:23], op=A.subtract)
            nc.scalar.activation(dy[:], dy[:], AF.Abs, bias=0.0, scale=0.5)
            sy = small.tile([P, JG * 8], f32, tag="sy")
            nc.gpsimd.tensor_tensor(
                sy[:].rearrange("p (jg c) -> p jg c", jg=JG),
                pv[:, :, 0:8], pv[:, :, 15:23], op=A.add)
            nc.gpsimd.tensor_scalar(sy[:], sy[:], 0.5, None, op0=A.mult)
            nc.gpsimd.tensor_tensor(
                pv[:, :, 0:8], sy[:].rearrange("p (jg c) -> p jg c", jg=JG),
                dy[:].rearrange("p (jg c) -> p jg c", jg=JG), op=A.add)

            gs = small.tile([P, JG], f32, tag="gs")
            nc.vector.tensor_reduce(
                gs[:].rearrange("p (j g) -> p j g", j=TB),
                pcat[:].rearrange("p (jg c) -> p jg c", jg=JG)[:, :, 0:15]
                .rearrange("p (j g) c -> p j g c", j=TB),
                axis=AX.X, op=A.max)

            gm8 = small.tile([P, TB * 8], f32, tag="gm8")
            for j in range(TB):
                nc.vector.max(out=gm8[:, j * 8:(j + 1) * 8],
                              in_=gs[:, j * G:(j + 1) * G])

            tg = gm8[:, 3::8]  # [P, TB]
            cmp = small.tile([P, TB * G], f32, tag="cmp")
            nc.vector.tensor_tensor(
                out=cmp[:].rearrange("p (j g) -> p j g", j=TB),
                in0=gs[:].rearrange("p (j g) -> p j g", j=TB),
                in1=tg.to_broadcast([P, TB, G]),
                op=A.is_lt)
            goff = small.tile([P, TB * G], f32, tag="goff")
            nc.gpsimd.tensor_scalar(goff[:], cmp[:], NEG, None, op0=A.mult)

            # swbm = swb + goff (in place; masked groups -> -1e30)
            for j in range(0, TB, SPLIT):
                sl = slice(j * E, j * E + SW)
                nc.gpsimd.tensor_add(
                    swb[:, sl].rearrange("p (j g e) -> p j g e", j=SPLIT, g=G),
                    swb[:, sl].rearrange("p (j g e) -> p j g e", j=SPLIT, g=G),
                    goff[:, j * G:(j + SPLIT) * G]
                    .rearrange("p (j g) -> p j g", j=SPLIT)
                    .to_broadcast([P, SPLIT, G, EG]))
            return scores, swb

        def phase_a2(i, scores, swb):
            v8b = small.tile([P, TB * 8], f32, tag="v8b")
            for j in range(TB):
                nc.vector.max(out=v8b[:, j * 8:(j + 1) * 8],
                              in_=swb[:, j * E:(j + 1) * E])

            c = 1.5 * 2.0 ** -23
            t8lo = small.tile([P, TB], f32, tag="t8lo")
            nc.vector.tensor_scalar(t8lo[:], v8b[:, 7::8], c - 1.0, None, op0=A.mult)
            t8hi = small.tile([P, TB], f32, tag="t8hi")
            nc.vector.tensor_scalar(t8hi[:], v8b[:, 7::8], -c - 1.0, None, op0=A.mult)
            nt8p = small.tile([P, TB], f32, tag="nt8p")
            nc.vector.tensor_tensor(nt8p[:], t8lo[:], t8hi[:], op=A.max)

            for j in range(TB):
                nc.scalar.activation(
                    swb[:, j * E:(j + 1) * E], swb[:, j * E:(j + 1) * E],
                    AF.Sign, bias=nt8p[:, j:j + 1], scale=1.0)

            for ci, j in enumerate(range(0, TB, SPLIT)):
                sl = slice(j * E, j * E + SW)
                if ci < SM_DVE_CHUNKS:
                    nc.vector.tensor_tensor(scores[:, sl], scores[:, sl],
                                            swb[:, sl], op=A.mult)
                else:
                    nc.gpsimd.tensor_tensor(scores[:, sl], scores[:, sl],
                                            swb[:, sl], op=A.mult)
            return scores

        def phase_b(i, scores):
            rows = slice(i * P * TB, (i + 1) * P * TB)
            v8u = small.tile([P, TB * 8], f32, tag="v8u")
            for j in range(TB):
                nc.vector.max(out=v8u[:, j * 8:(j + 1) * 8],
                              in_=scores[:, j * E:(j + 1) * E])

            idx8 = small.tile([P, TB * 8], mybir.dt.int32, tag="idx8")
            for j in range(TB):
                nc.vector.max_index(
                    out=idx8[:, j * 8:(j + 1) * 8].bitcast(mybir.dt.uint32),
                    in_max=v8u[:, j * 8:(j + 1) * 8],
                    in_values=scores[:, j * E:(j + 1) * E])

            ssum = small.tile([P, TB], f32, tag="ssum")
            nc.vector.tensor_reduce(ssum[:],
                                    v8u[:].rearrange("p (j k) -> p j k", j=TB),
                                    axis=AX.X, op=A.add)
            ssum4 = small.tile([P, TB], f32, tag="ssum4")
            nc.scalar.mul(ssum4[:], ssum[:], 0.4)
            rec = small.tile([P, TB], f32, tag="rec")
            nc.vector.reciprocal(rec[:], ssum4[:])

            vals8 = small.tile([P, TB * 8], f32, tag="vals8")
            nc.gpsimd.tensor_tensor(
                vals8[:].rearrange("p (j k) -> p j k", j=TB),
                v8u[:].rearrange("p (j k) -> p j k", j=TB),
                rec[:].to_broadcast([P, TB, 8]), op=A.mult)

            oi = idx_d[rows, :].rearrange("(j p) k -> p j k", p=P)
            ov = vals_d[rows, :].rearrange("(j p) k -> p j k", p=P)
            nc.sync.dma_start(
                oi, idx8[:].rearrange("p (j k) -> p j k", j=TB))
            nc.sync.dma_start(ov, vals8[:].rearrange("p (j k) -> p j k", j=TB))

        prev = None
        for i in range(n_groups):
            sc, sw = phase_a1(i)
            if prev is not None:
                phase_b(i - 1, prev)
            prev = phase_a2(i, sc, sw)
        phase_b(n_groups - 1, prev)

    nc.compile()
    return nc


_NC_CACHE = {}


def _get_nc(n_tokens: int, w2=None):
    if n_tokens not in _NC_CACHE:
        _NC_CACHE[n_tokens] = build_bass(n_tokens)
    return _NC_CACHE[n_tokens]


def _host_tiles(bias):
    biasb = np.ascontiguousarray(
        np.broadcast_to(np.tile(bias, TB)[None, :], (P, TB * E)).astype(np.float32))
    return biasb, None, None


def run_spmd(nc, logits, biasb, revb=None, trace=False):
    from concourse import bass_utils

    n = logits.shape[0] // N_CORES
    in_maps = [
        {"logits": np.ascontiguousarray(logits[c * n:(c + 1) * n]),
         "biasb": biasb}
        for c in range(N_CORES)
    ]
    res = bass_utils.run_bass_kernel_spmd(nc, in_maps, list(range(N_CORES)),
                                          trace=trace)
    idx = np.concatenate([r["idx"] for r in res.results], axis=0)
    vals = np.concatenate([r["vals"] for r in res.results], axis=0)
    return (idx.astype(np.int32), vals.astype(np.float32)), res


def make_in_maps(logits, bias):
    biasb, _, _ = _host_tiles(bias)
    n = logits.shape[0] // N_CORES
    return [
        {"logits": np.ascontiguousarray(logits[c * n:(c + 1) * n]),
         "biasb": biasb}
        for c in range(N_CORES)
    ]


def kernel(logits, e_score_correction_bias):
    logits = np.asarray(logits, dtype=np.float32)
    bias = np.asarray(e_score_correction_bias, dtype=np.float32)
    assert logits.shape == (T_FULL, E)
    biasb, revb, _ = _host_tiles(bias)
    nc = _get_nc(T_CORE)
    (idx, vals), _ = run_spmd(nc, logits, biasb, revb)
    return idx, vals

